# revision 36
# baseline (speedup 1.0000x reference)
"""Multi-head self-attention (B=2, N=4096, D=768, H=12, dh=64) on 8 trn2 NeuronCores.

Sharding: core c handles batch b=c//4 and heads 3*(c%4)..3*(c%4)+2 (head-parallel
attention); an AllGather per 512-token i-chunk redistributes head-outputs so each
core projects its own 192 output columns for all tokens (column-parallel output
projection), assembled host-side.

Per-core pipeline (all matmuls fp16 except psum accumulate):
  x is shipped fp16 and kept resident in SBUF (48KB/partition), so projections
  read it directly with 1024-wide moving operands and no per-pass DMA.
  pass 1: project k (pre-scaled by softmax_scale/8 host-side) and v for all
    tokens; k kept d-on-partition fp16 in row-tiled head-pair layout, v
    PE-transposed into v_sb as [v_h | ones64] fp16 blocks per 128-key chunk —
    the ones columns make each PV matmul emit the softmax denominator
    replicated across psum partitions 64:128 for free.
  pass 2 (interleaved with attention): q projected per 512-token chunk.
  attention per (i-chunk, key-chunk): row-tiled QK pair (concurrent PE
    row-strips) -> psum u = s/8 -> exp(8u) split ~11:5 between ACT table exp
    and a 2-op DVE polynomial (taylor4(u), x^8), DVE batches interleaved so
    the PE never idles a full HAM window -> fp16 PV with [v|ones] stationary
    -> reciprocal[64,512] + multiply -> fp16 AllGather -> column-parallel
    output projection with DVE bias add.
"""
import sys

sys.path.insert(0, "/opt/trn_rl_repo")

import numpy as np

import concourse.bass as bass
import concourse.mybir as mybir
import concourse.tile as tile
import concourse.bacc as bacc
from concourse.masks import make_identity

N_CORES = 8
B, N, D, H, DH = 2, 4096, 768, 12, 64
HPC = 3            # heads per core
SCALE = D ** -0.5
F32 = mybir.dt.float32
F16 = mybir.dt.float16
AF = mybir.ActivationFunctionType
IC = 512           # query chunk size
NIC = N // IC      # 8 i-chunks
VW = 384           # v_sb cols per 128-key chunk: [v0|1*64|v1|1*64|v2|1*64]
EXP_GROUP = 32     # exp batches: (i*EXP_DVE) % EXP_GROUP < EXP_DVE -> DVE path
EXP_DVE = 10       # ~31% of batches on DVE, evenly interleaved


# ---------------------------------------------------------------- custom DVE exp
def _register_exp_ops():
    """exp(8u) as two DVE ops: EXP_P4_ANT = taylor4(u); EXP_SQ8_ANT = x^8."""
    import concourse.dve_ops as dve_ops
    from concourse.dve_ops import DveOp, OPS, CUSTOM_DVE_SPECS, _SUB_OPCODE_FOR_NAME
    from concourse.dve_spec import Spec, Src0, C0, C1, C2, One, sq, lower
    from concourse.dve_uop import DveOpSpec

    if "EXP_P4_ANT" in _SUB_OPCODE_FOR_NAME:
        return dve_ops.EXP_P4_ANT, dve_ops.EXP_SQ8_ANT

    u = Src0
    p4 = ((((u * C0) + C1) * u + C2) * u + One) * u + One  # c0=1/24 c1=1/6 c2=1/2
    spec_p4 = Spec(
        body=p4,
        reference=lambda in0, in1, s0, s1, imm2: (
            (((in0 * s0 + s1) * in0 + imm2) * in0 + 1.0) * in0 + 1.0
        ),
    )
    spec_sq8 = Spec(
        body=sq(sq(sq(Src0))),
        reference=lambda in0, in1, s0, s1, imm2: in0 ** 8,
    )

    def _mk(name, spec):
        opcode = max(_SUB_OPCODE_FOR_NAME.values()) + 1
        _SUB_OPCODE_FOR_NAME[name] = opcode
        shas = {}
        for ver in ("v3", "v4"):
            s = DveOpSpec(
                name=name, opcode=opcode, uops=lower(spec, ver=ver), rd1_en=False
            )
            shas[ver] = s.sha(ver)
        op = DveOp(name, spec, subdim=False, uops_sha=shas)
        OPS.append(op)
        CUSTOM_DVE_SPECS[name] = spec
        setattr(dve_ops, name, op)
        return op

    p4_op = _mk("EXP_P4_ANT", spec_p4)
    sq8_op = _mk("EXP_SQ8_ANT", spec_sq8)
    return p4_op, sq8_op


# ---------------------------------------------------------------- program build
_PROG_CACHE = {}


def build_program(use_dve_exp=True):
    key = ("prog", use_dve_exp)
    if key in _PROG_CACHE:
        return _PROG_CACHE[key]
    p4_op, sq8_op = _register_exp_ops()

    nc = bacc.Bacc("TRN2", target_bir_lowering=False, debug=False, num_devices=N_CORES)

    BF16 = mybir.dt.bfloat16
    xT = nc.dram_tensor("xT", [D, N], BF16, kind="ExternalInput").ap()
    wqkv = nc.dram_tensor("wqkv", [D, 768], BF16, kind="ExternalInput").ap()
    bqkv = nc.dram_tensor("bqkv", [6, 128], F32, kind="ExternalInput").ap()
    wout = nc.dram_tensor("wout", [D, 192], F16, kind="ExternalInput").ap()
    bout = nc.dram_tensor("bout", [2, 128], F32, kind="ExternalInput").ap()
    y = nc.dram_tensor("y", [HPC * DH, N], F32, kind="ExternalOutput").ap()

    with tile.TileContext(nc, trace_sim=False) as tc:
        with (
            tc.tile_pool(name="consts", bufs=1) as consts,
            tc.tile_pool(name="persist", bufs=1) as persist,
            tc.tile_pool(name="otp", bufs=2) as otp,
            tc.tile_pool(name="spsum", bufs=3, space="PSUM") as spsum,
            tc.tile_pool(name="opsum", bufs=1, space="PSUM") as opsum,
            tc.tile_pool(name="dram", bufs=1, space="DRAM") as dram,
        ):
            # ---------------- constants + resident fp16 x
            ident_f = consts.tile([128, 128], F32)
            make_identity(nc, ident_f[:])
            ident16 = consts.tile([128, 128], F16)
            nc.scalar.copy(ident16[:], ident_f[:])

            # resident fp16 x, one tile per 1024-token block so early
            # projections only wait on their own block's DMAs
            x16_t = [
                persist.tile([128, 6 * 1024], BF16, tag=f"x16_{nq}",
                             name=f"x16_{nq}")
                for nq in range(4)
            ]
            for nq in range(4):
                for fc in range(6):
                    for pb in range(2):
                        nc.sync.dma_start(
                            out=x16_t[nq][64 * pb : 64 * (pb + 1),
                                          fc * 1024 : (fc + 1) * 1024],
                            in_=xT[fc * 128 + 64 * pb : fc * 128 + 64 * (pb + 1),
                                   nq * 1024 : (nq + 1) * 1024],
                        )
            wq_sb = consts.tile([128, 6 * 768], BF16)  # 6 f-chunks of [128,768]
            for fc in range(6):
                nc.sync.dma_start(
                    out=wq_sb[:, fc * 768 : (fc + 1) * 768],
                    in_=wqkv[fc * 128 : (fc + 1) * 128, :],
                )
            bq_sb = consts.tile([128, 6], F32)  # per-m-chunk bias columns
            for m in range(6):
                nc.sync.dma_start(
                    out=bq_sb[:, m : m + 1],
                    in_=bqkv[m : m + 1, :].rearrange("a p -> p a"),
                )

            # ---------------- persistent activations (all fp16)
            # q tiles are per-i-chunk so projecting chunk ic+1 mid-attention
            # carries no tile-granularity WAR against chunk ic's reads
            qT01_t = [
                persist.tile([128, IC], F16, tag=f"q01_{i}", name=f"q01_{i}")
                for i in range(NIC)
            ]
            qT2_t = [
                persist.tile([128, IC], F16, tag=f"q2_{i}", name=f"q2_{i}")
                for i in range(NIC)
            ]
            kT01 = persist.tile([128, N], F16)  # scaled kT, heads 0/1
            kT2 = persist.tile([128, N], F16)   # scaled kT head2, duplicated
            v_sb = persist.tile([128, 32 * VW], F16)
            v_view = v_sb[:].rearrange("p (j c) -> p j c", c=VW)
            for h in range(HPC):
                nc.vector.memset(v_view[:, :, h * 128 + 64 : h * 128 + 128], 1.0)

            # AllGather eighths: in [192, 512] -> out [768, 512] (rank-major rows)
            oT_q = [dram.tile([HPC * DH, IC], F16, name=f"oT_q{i}") for i in range(8)]
            ag_q = [dram.tile([D, IC], F16, name=f"ag_q{i}") for i in range(8)]

            # output projection weights (phase 3 is emitted inline per i-chunk)
            wo_sb = consts.tile([128, 6 * 192], F16)  # w_out col-slice, 6 d-chunks
            for dc in range(6):
                nc.sync.dma_start(
                    out=wo_sb[:, dc * 192 : (dc + 1) * 192],
                    in_=wout[dc * 128 : (dc + 1) * 128, :],
                )
            bo_sb = consts.tile([128, 2], F32)
            for m in range(2):
                nc.sync.dma_start(
                    out=bo_sb[:, m : m + 1],
                    in_=bout[m : m + 1, :].rearrange("a p -> p a"),
                )

            with tc.tile_pool(name="work", bufs=2) as work, \
                 tc.tile_pool(name="ptp", bufs=3) as ptp, \
                 tc.tile_pool(name="ph3", bufs=1) as ph3:

                def proj_chunk(m, tsl, out_psum):
                    """fp16 matmul of m-chunk over token slice tsl into psum."""
                    nq, off = tsl.start // 1024, tsl.start % 1024
                    w = tsl.stop - tsl.start
                    for fc in range(6):
                        nc.tensor.matmul(
                            out_psum,
                            wq_sb[:, fc * 768 + m * 128 : fc * 768 + (m + 1) * 128],
                            x16_t[nq][:, fc * 1024 + off : fc * 1024 + off + w],
                            start=(fc == 0),
                            stop=(fc == 5),
                        )

                # ---------------- pass 1: k + v projection (q deferred)
                for tau in range(8):
                    ts = slice(tau * IC, (tau + 1) * IC)
                    for m, dst in ((2, kT01), (3, kT2)):
                        sp = spsum.tile([128, 1024], F32, tag="sp", bufs=3)
                        proj_chunk(m, ts, sp[:, 0:IC])
                        nc.scalar.activation(
                            dst[:, ts], sp[:, 0:IC], AF.Identity,
                            bias=bq_sb[:, m : m + 1],
                        )
                    vt01 = work.tile([128, IC], F16, tag="vt01")
                    vt2 = work.tile([64, IC], F16, tag="vt2")
                    sp = spsum.tile([128, 1024], F32, tag="sp", bufs=3)
                    proj_chunk(4, ts, sp[:, 0:IC])
                    nc.scalar.activation(
                        vt01[:], sp[:, 0:IC], AF.Identity, bias=bq_sb[:, 4:5]
                    )
                    sp = spsum.tile([128, 1024], F32, tag="sp", bufs=3)
                    proj_chunk(5, ts, sp[:, 0:IC])
                    nc.scalar.activation(
                        vt2[:], sp[0:64, 0:IC], AF.Identity, bias=bq_sb[0:64, 5:6]
                    )
                    # transpose v into token-major [v|ones] layout, 4 t-blocks
                    # per psum tile: [v01(t0..t3) 512 | v2(t0..t3) 256]
                    pv = spsum.tile([128, 2048], F16, tag="sp", bufs=3)
                    for s in range(4):
                        nc.tensor.transpose(
                            pv[:, s * 128 : (s + 1) * 128],
                            vt01[:, s * 128 : (s + 1) * 128],
                            ident16[:],
                        )
                        nc.tensor.transpose(
                            pv[:, 512 + s * 64 : 512 + (s + 1) * 64],
                            vt2[0:64, s * 128 : (s + 1) * 128],
                            ident16[0:64, 0:64],
                        )
                    jc0 = 4 * tau
                    pv01 = pv[:, 0:512].rearrange("p (t c) -> p t c", c=128)
                    pv2 = pv[:, 512:768].rearrange("p (t c) -> p t c", c=64)
                    nc.vector.tensor_copy(
                        out=v_view[:, jc0 : jc0 + 4, 0:64], in_=pv01[:, :, 0:64]
                    )
                    nc.vector.tensor_copy(
                        out=v_view[:, jc0 : jc0 + 4, 128:192],
                        in_=pv01[:, :, 64:128],
                    )
                    nc.vector.tensor_copy(
                        out=v_view[:, jc0 : jc0 + 4, 256:320], in_=pv2[:, :, :]
                    )

                # ---------------- pass 2: q projection + attention per i-chunk
                exp_batch_idx = [0]

                def exp_batch(sp):
                    """exp(8u) on a [128,1024] psum batch -> fp16 SBUF tile."""
                    i = exp_batch_idx[0]
                    exp_batch_idx[0] += 1
                    pt = ptp.tile([128, 1024], F16, tag="pt", bufs=5)
                    if use_dve_exp and ((i * EXP_DVE) % EXP_GROUP) < EXP_DVE:
                        tmp = ptp.tile([128, 1024], F16, tag="exptmp", bufs=2)
                        nc.vector._custom_dve(
                            p4_op, out=tmp[:], in0=sp[:],
                            s0=1.0 / 24, s1=1.0 / 6, imm2=0.5,
                        )
                        nc.vector._custom_dve(sq8_op, out=pt[:], in0=tmp[:])
                    else:
                        nc.scalar.activation(pt[:], sp[:], AF.Exp, scale=8.0)
                    return pt

                def q_proj_half(ic, m):
                    isl = slice(ic * IC, (ic + 1) * IC)
                    dst = qT01_t[ic] if m == 0 else qT2_t[ic]
                    pp = spsum.tile([128, 1024], F32, tag="sp", bufs=3,
                                    name=f"pp{m}_{ic}")
                    proj_chunk(m, isl, pp[:, 0:IC])
                    nc.scalar.activation(
                        dst[:], pp[:, 0:IC], AF.Identity,
                        bias=bq_sb[:, m : m + 1],
                    )

                def q_proj(ic):
                    q_proj_half(ic, 0)
                    q_proj_half(ic, 1)

                def norm_store(po_h, ic, h):
                    """o_h = num / l. ACT copies l down to partition base 0
                    (ACT handles cross-base; custom DVE ops do not), then the
                    fast approx reciprocal and the multiply run base-aligned."""
                    l0 = otp.tile([64, IC], F32, tag="l0", bufs=2)
                    nc.scalar.copy(l0[:], po_h[64:128, :])
                    rr = otp.tile([64, IC], F32, tag="rr", bufs=2)
                    nc.vector.reciprocal_approx_fast(rr[:], l0[:])
                    ot = otp.tile([64, IC], F16, tag="ot", bufs=3)
                    nc.vector.tensor_tensor(
                        out=ot[:], in0=po_h[0:64, :], in1=rr[:],
                        op=mybir.AluOpType.mult,
                    )
                    nc.sync.dma_start(
                        out=oT_q[ic][64 * h : 64 * h + 64, :], in_=ot[:]
                    )

                ogs_t = {}

                def og_load(qtr):
                    agr = ag_q[qtr][:]
                    ogs = []
                    for dc in range(6):
                        og = ph3.tile([128, IC], F16, tag="og", bufs=12,
                                      name=f"og{dc}_{qtr}")
                        nc.sync.dma_start(
                            out=og[:], in_=agr[dc * 128 : (dc + 1) * 128, :]
                        )
                        ogs.append(og)
                    ogs_t[qtr] = ogs

                def phase3_ec(qtr, ec):
                    ogs = ogs_t[qtr]
                    elo, ew = ((0, 128), (128, 64))[ec]
                    if True:
                        py = spsum.tile([128, 1024], F32, tag="sp", bufs=3,
                                        name=f"py{ec}_{qtr}")
                        for dc in range(6):
                            nc.tensor.matmul(
                                py[0:ew, 0:IC],
                                wo_sb[:, dc * 192 + elo : dc * 192 + elo + ew],
                                ogs[dc][:],
                                start=(dc == 0), stop=(dc == 5),
                            )
                        ysb = ph3.tile([128, IC], F32, tag="ysb", bufs=3,
                                       name=f"ysb{ec}_{qtr}")
                        nc.scalar.activation(
                            ysb[0:ew, :], py[0:ew, 0:IC], AF.Identity,
                            bias=bo_sb[0:ew, ec : ec + 1],
                        )
                        nc.sync.dma_start(
                            out=y[elo : elo + ew, qtr * IC : (qtr + 1) * IC],
                            in_=ysb[0:ew, :],
                        )

                q_proj(0)

                # ---- flat software-pipelined batch stream across all
                # i-chunks: QK(b)+exp(b) always emitted before PV(b-1), so
                # the in-order PE queue and the sp ring never drain at
                # chunk boundaries. Bookkeeping (norms, q-proj, AllGather,
                # output projection) is emitted as in-stream hooks.
                po_t = {}

                def qk(ic, kind, idx):
                    isl = slice(ic * IC, (ic + 1) * IC)
                    sp = spsum.tile([128, 1024], F32, tag="sp", bufs=3,
                                    name=f"sp{kind}_{ic}_{idx}")
                    if kind == "01":
                        nc.tensor.matmul(
                            sp[:, 0:IC],
                            kT01[0:64, idx * 128 : (idx + 1) * 128],
                            qT01_t[ic][0:64, :],
                            start=True, stop=True, tile_position=(0, 0),
                        )
                        nc.tensor.matmul(
                            sp[:, IC:1024],
                            kT01[64:128, idx * 128 : (idx + 1) * 128],
                            qT01_t[ic][64:128, :],
                            start=True, stop=True, tile_position=(64, 0),
                        )
                    else:
                        nc.tensor.matmul(
                            sp[:, 0:IC],
                            kT2[0:64, (2 * idx) * 128 : (2 * idx + 1) * 128],
                            qT2_t[ic][0:64, :],
                            start=True, stop=True, tile_position=(0, 0),
                        )
                        nc.tensor.matmul(
                            sp[:, IC:1024],
                            kT2[64:128, (2 * idx + 1) * 128 : (2 * idx + 2) * 128],
                            qT2_t[ic][64:128, :],
                            start=True, stop=True, tile_position=(64, 0),
                        )
                    return exp_batch(sp)

                def pv(ic, kind, idx, pt):
                    if kind == "01":
                        if idx == 0:
                            po_t[ic] = [
                                opsum.tile([128, IC], F32, tag="po0", bufs=1,
                                           name=f"po0_{ic}"),
                                opsum.tile([128, IC], F32, tag="po1", bufs=1,
                                           name=f"po1_{ic}"),
                            ]
                        po = po_t[ic]
                        nc.tensor.matmul(
                            po[0][:],
                            v_sb[:, idx * VW : idx * VW + 128],
                            pt[:, 0:IC],
                            start=(idx == 0), stop=(idx == 31),
                        )
                        nc.tensor.matmul(
                            po[1][:],
                            v_sb[:, idx * VW + 128 : idx * VW + 256],
                            pt[:, IC:1024],
                            start=(idx == 0), stop=(idx == 31),
                        )
                    else:
                        if idx == 0:
                            po_t[ic].append(
                                opsum.tile([128, IC], F32, tag="po0", bufs=1,
                                           name=f"po2_{ic}")
                            )
                        po = po_t[ic]
                        for s in range(2):
                            jc = 2 * idx + s
                            nc.tensor.matmul(
                                po[2][:],
                                v_sb[:, jc * VW + 256 : jc * VW + 384],
                                pt[:, s * IC : (s + 1) * IC],
                                start=(jc == 0), stop=(jc == 31),
                            )

                def post_pv_hooks(ic, kind, idx):
                    if kind == "01":
                        if idx == 6 and ic + 1 < NIC:
                            q_proj_half(ic + 1, 0)
                        elif idx == 14 and ic + 1 < NIC:
                            q_proj_half(ic + 1, 1)
                        elif idx == 31:
                            norm_store(po_t[ic][0], ic, 0)
                            norm_store(po_t[ic][1], ic, 1)
                    elif kind == "2" and idx == 0 and ic >= 1:
                        og_load(ic - 1)
                    elif kind == "2" and idx == 5 and ic >= 1:
                        phase3_ec(ic - 1, 0)
                    elif kind == "2" and idx == 11 and ic >= 1:
                        phase3_ec(ic - 1, 1)
                    elif kind == "2" and idx == 15:
                        norm_store(po_t[ic][2], ic, 2)
                        nc.gpsimd.collective_compute(
                            "AllGather",
                            mybir.AluOpType.bypass,
                            replica_groups=[[0, 1, 2, 3], [4, 5, 6, 7]],
                            ins=[oT_q[ic][:]],
                            outs=[ag_q[ic][:]],
                        )

                stream = [
                    (ic, kind, idx)
                    for ic in range(NIC)
                    for kind, count in (("01", 32), ("2", 16))
                    for idx in range(count)
                ]
                from collections import deque
                pending = deque()
                for b in stream:
                    pt = qk(*b)
                    pending.append((b, pt))
                    if len(pending) > 2:
                        done = pending.popleft()
                        pv(*done[0], done[1])
                        post_pv_hooks(*done[0])
                while pending:
                    done = pending.popleft()
                    pv(*done[0], done[1])
                    post_pv_hooks(*done[0])
                og_load(NIC - 1)
                phase3_ec(NIC - 1, 0)
                phase3_ec(NIC - 1, 1)

    nc.compile()
    _PROG_CACHE[key] = nc
    return nc


# ---------------------------------------------------------------- host wrapper
def make_in_maps(x, w_qkv, b_qkv, w_out, b_out):
    """Build the 8 per-core input dicts from full inputs."""
    in_maps = []
    import ml_dtypes
    xTb = [np.ascontiguousarray(x[b].T.astype(ml_dtypes.bfloat16)) for b in range(B)]
    kscale = np.float32(SCALE / 8.0)
    for c in range(N_CORES):
        b = c // 4
        hs = HPC * (c % 4)

        def sect(kind, h):  # q=0,k=1,v=2
            lo = kind * (H * DH) + h * DH
            return w_qkv[:, lo : lo + DH], b_qkv[lo : lo + DH]

        q0, bq0 = sect(0, hs); q1, bq1 = sect(0, hs + 1); q2, bq2 = sect(0, hs + 2)
        k0, bk0 = sect(1, hs); k1, bk1 = sect(1, hs + 1); k2, bk2 = sect(1, hs + 2)
        v0, bv0 = sect(2, hs); v1, bv1 = sect(2, hs + 1); v2, bv2 = sect(2, hs + 2)
        z = np.zeros_like(q2); bz = np.zeros_like(bq2)
        # m-chunks: [q0|q1], [q2|q2], [k0|k1]*s, [k2|k2]*s, [v0|v1], [v2|0]
        cols = np.concatenate(
            [q0, q1, q2, q2, k0 * kscale, k1 * kscale, k2 * kscale, k2 * kscale,
             v0, v1, v2, z], axis=1).astype(ml_dtypes.bfloat16)
        bias = np.concatenate(
            [bq0, bq1, bq2, bq2, bk0 * kscale, bk1 * kscale, bk2 * kscale,
             bk2 * kscale, bv0, bv1, bv2, bz]).astype(np.float32)
        q = c % 4
        bo = np.zeros((2, 128), np.float32)
        bo[0, :] = b_out[192 * q : 192 * q + 128]
        bo[1, :64] = b_out[192 * q + 128 : 192 * q + 192]
        in_maps.append({
            "xT": xTb[b],
            "wqkv": np.ascontiguousarray(cols),
            "bqkv": np.ascontiguousarray(bias.reshape(6, 128)),
            "wout": np.ascontiguousarray(
                w_out[:, 192 * q : 192 * (q + 1)].astype(np.float16)),
            "bout": bo,
        })
    return in_maps


def assemble_output(results):
    out = np.empty((B, N, D), dtype=np.float32)
    for c in range(N_CORES):
        b = c // 4
        q = c % 4
        out[b, :, 192 * q : 192 * (q + 1)] = results[c]["y"].T
    return out


def kernel(x, w_qkv, b_qkv, w_out, b_out):
    from concourse.bass_utils import run_bass_kernel_spmd

    x = np.asarray(x, dtype=np.float32)
    nc = build_program()
    in_maps = make_in_maps(
        x, np.asarray(w_qkv, np.float32), np.asarray(b_qkv, np.float32),
        np.asarray(w_out, np.float32), np.asarray(b_out, np.float32))
    res = run_bass_kernel_spmd(nc, in_maps, core_ids=list(range(N_CORES)))
    return assemble_output(res.results)
